# revision 4
# baseline (speedup 1.0000x reference)
"""Single-query attention (decode step) on 8 Trainium2 NeuronCores.

Problem: outputs [16, 4096, 2048] f32 (K = V), last_h [16, 2048] f32 (q).
Returns (ctx [16, 2048], attn [16, 4096]).

Sharding: batch dim 16 -> 2 batches per core (data parallel, no collectives).
Per core, per batch: stream K tiles [128, CHUNK_T, 2048] from HBM once;
  - QK^T: DVE tensor_tensor_reduce (fused multiply + free-dim reduce) against a
    partition-broadcast copy of q  -> logits [128, 32]
  - softmax without max-subtraction (logits are O(5) for this problem; exp is
    safe in fp32), denominator via ones-vector matmul partition reduction
  - AV: TensorE matmul, p column [128,1] stationary (float32r => full rate),
    K tile moving, accumulated over the 32 n-tiles in PSUM [1, 2048]
"""

import numpy as np

B, N, H = 16, 4096, 2048
NCORES = 8
BPC = B // NCORES        # batches per core
P = 128                  # partitions
NT = N // P              # n-tiles per batch (32)
CHUNK_T = 4              # n-tiles per DMA chunk
NCHUNK = NT // CHUNK_T
HB = 512                 # AV matmul moving free-dim (one PSUM bank of fp32)
HT = H // HB             # h-chunks for AV (4)
SCALE = float(1.0 / np.sqrt(np.float32(H), dtype=np.float32))

_compiled_nc = None


def _build():
    import concourse.bass as bass
    import concourse.bacc as bacc
    import concourse.tile as tile
    from concourse import mybir

    f32 = mybir.dt.float32
    f32r = mybir.dt.float32r

    nc = bacc.Bacc("TRN2", target_bir_lowering=False, debug=False,
                   num_devices=NCORES)

    kv = nc.dram_tensor("kv", [BPC, N, H], f32, kind="ExternalInput")
    q = nc.dram_tensor("q", [BPC, H], f32, kind="ExternalInput")
    ctx_out = nc.dram_tensor("ctx", [BPC, H], f32, kind="ExternalOutput")
    attn_out = nc.dram_tensor("attn", [BPC, N], f32, kind="ExternalOutput")

    with tile.TileContext(nc) as tc:
        with (
            tc.tile_pool(name="kpool", bufs=3) as kpool,
            tc.tile_pool(name="spool", bufs=2) as spool,
            tc.tile_pool(name="s2pool", bufs=2) as s2pool,
            tc.tile_pool(name="qpool", bufs=2) as qpool,
            tc.tile_pool(name="small", bufs=2) as small,
            tc.tile_pool(name="consts", bufs=1) as consts,
            tc.tile_pool(name="pctx", bufs=1, space="PSUM") as pctx,
            tc.tile_pool(name="psmall", bufs=2, space="PSUM") as psmall,
        ):
            ones_col = consts.tile([P, 1], f32)
            nc.vector.memset(ones_col, 1.0)
            ones_row = consts.tile([1, P], f32)
            nc.vector.memset(ones_row, 1.0)

            for b in range(BPC):
                # q[b] broadcast to all 128 partitions: [128, H]
                qb = qpool.tile([P, H], f32)
                q_src = q[b]
                q_bc = bass.AP(tensor=q_src.tensor, offset=q_src.offset,
                               ap=[[0, P]] + list(q_src.ap))
                nc.gpsimd.dma_start(out=qb, in_=q_bc)

                logits = small.tile([P, NT], f32)
                pexp = small.tile([P, NT], f32)
                pexp_r = small.tile([P, NT], f32r)
                ctx_ps = pctx.tile([1, H], mybir.dt.float32)

                for c in range(NCHUNK):
                    # float32r tile (cast during DMA, gpsimd/SWDGE path) so the
                    # full-rate f32r matmul sees a properly "rounded" producer
                    kt = kpool.tile([P, CHUNK_T, H], f32r)
                    src = kv[b, c * CHUNK_T * P:(c + 1) * CHUNK_T * P, :]
                    src = src.rearrange("(t p) h -> p t h", p=P)
                    nc.gpsimd.dma_start(out=kt, in_=src)

                    for t in range(CHUNK_T):
                        i = c * CHUNK_T + t
                        # logits[:, i] = sum_h kt[:, t, h] * qb[:, h] * SCALE
                        # (DVE multiply, then ACT accumulate-reduce)
                        scr = spool.tile([P, H], f32)
                        nc.vector.tensor_mul(scr, kt[:, t, :].bitcast(f32), qb)
                        scr2 = s2pool.tile([P, H], f32)
                        nc.scalar.activation(
                            out=scr2,
                            in_=scr,
                            func=mybir.ActivationFunctionType.Identity,
                            scale=SCALE,
                            accum_out=logits[:, i:i + 1],
                        )

                    i0 = c * CHUNK_T
                    nc.scalar.activation(
                        out=pexp[:, i0:i0 + CHUNK_T],
                        in_=logits[:, i0:i0 + CHUNK_T],
                        func=mybir.ActivationFunctionType.Exp,
                    )
                    nc.scalar.activation(
                        out=pexp_r[:, i0:i0 + CHUNK_T],
                        in_=logits[:, i0:i0 + CHUNK_T],
                        func=mybir.ActivationFunctionType.Exp,
                    )
                    for t in range(CHUNK_T):
                        i = c * CHUNK_T + t
                        # ctx_ps[0, :] += pexp[:, i].T @ kt[:, t, :]
                        for j in range(HT):
                            nc.tensor.matmul(
                                ctx_ps[0:1, j * HB:(j + 1) * HB],
                                pexp_r[:, i:i + 1],
                                kt[:, t, j * HB:(j + 1) * HB],
                                start=(i == 0),
                                stop=(i == NT - 1),
                            )

                # softmax denominator: total = sum over all 4096 exps
                s1 = small.tile([P, 1], f32)
                nc.vector.reduce_sum(out=s1, in_=pexp,
                                     axis=mybir.AxisListType.X)
                tot_ps = psmall.tile([1, 1], mybir.dt.float32)
                nc.tensor.matmul(tot_ps, ones_col, s1, start=True, stop=True)
                tot_sb = small.tile([1, 1], f32)
                nc.scalar.copy(out=tot_sb, in_=tot_ps)
                # broadcast total to 128 partitions via rank-1 matmul
                totbc_ps = psmall.tile([P, 1], mybir.dt.float32)
                nc.tensor.matmul(totbc_ps, ones_row, tot_sb,
                                 start=True, stop=True)
                invbc = small.tile([P, 1], f32)
                nc.vector.reciprocal(invbc, totbc_ps)

                # ctx = ctx_ps * inv
                ctx_sb = small.tile([1, H], f32)
                nc.scalar.mul(out=ctx_sb, in_=ctx_ps, mul=invbc[0:1, :])
                nc.sync.dma_start(out=ctx_out[b:b + 1, :], in_=ctx_sb)

                # attn: transpose pexp [128, 32] -> [32, 128], scale, store
                pT = small.tile([NT, P], f32)
                for k in range(P // NT):
                    nc.vector.transpose(
                        out=pT[:, k * NT:(k + 1) * NT],
                        in_=pexp[k * NT:(k + 1) * NT, :],
                    )
                nc.vector.tensor_scalar_mul(pT, pT, invbc[0:NT, :])
                attn_view = attn_out[b].rearrange("(a m) -> a m", m=P)
                nc.sync.dma_start(out=attn_view, in_=pT)

    nc.compile()
    return nc


def _get_nc():
    global _compiled_nc
    if _compiled_nc is None:
        _compiled_nc = _build()
    return _compiled_nc


def kernel(outputs: np.ndarray, last_h: np.ndarray):
    from concourse.bass_utils import run_bass_kernel_spmd

    nc = _get_nc()
    outputs = np.asarray(outputs, dtype=np.float32)
    last_h = np.asarray(last_h, dtype=np.float32)

    in_maps = [
        {"kv": outputs[c * BPC:(c + 1) * BPC],
         "q": last_h[c * BPC:(c + 1) * BPC]}
        for c in range(NCORES)
    ]
    res = run_bass_kernel_spmd(nc, in_maps, list(range(NCORES)))
    ctx = np.concatenate([res.results[c]["ctx"] for c in range(NCORES)], axis=0)
    attn = np.concatenate([res.results[c]["attn"] for c in range(NCORES)],
                          axis=0)
    return ctx, attn


# revision 7
# speedup vs baseline: 33.9697x; 33.9697x over previous
"""Single-query attention (decode step) on 8 Trainium2 NeuronCores.

Problem: outputs [16, 4096, 2048] f32 (K = V), last_h [16, 2048] f32 (q).
Returns (ctx [16, 2048], attn [16, 4096]).

Sharding: batch dim 16 -> 2 batches per core (data parallel, no collectives).
Per core, per batch: stream K tiles [128, CHUNK_T, 2048] from HBM once;
  - QK^T: DVE tensor_tensor_reduce (fused multiply + free-dim reduce) against a
    partition-broadcast copy of q  -> logits [128, 32]
  - softmax without max-subtraction (logits are O(5) for this problem; exp is
    safe in fp32), denominator via ones-vector matmul partition reduction
  - AV: TensorE matmul, p column [128,1] stationary (float32r => full rate),
    K tile moving, accumulated over the 32 n-tiles in PSUM [1, 2048]
"""

import numpy as np

B, N, H = 16, 4096, 2048
NCORES = 8
BPC = B // NCORES        # batches per core
P = 128                  # partitions
NT = N // P              # n-tiles per batch (32)
CHUNK_T = 4              # n-tiles per DMA chunk
NCHUNK = NT // CHUNK_T
HB = 512                 # AV matmul moving free-dim (one PSUM bank of fp32)
HT = H // HB             # h-chunks for AV (4)
SCALE = float(1.0 / np.sqrt(np.float32(H), dtype=np.float32))

_compiled_nc = None


def _build(repeat: int = 1):
    """Build + compile the per-core program. repeat>1 wraps the whole body in
    an on-device For loop (used only for benchmarking; amortizes the host
    dispatch overhead so device time can be measured)."""
    import contextlib
    import concourse.bass as bass
    import concourse.bacc as bacc
    import concourse.tile as tile
    from concourse import mybir

    f32 = mybir.dt.float32
    f32r = mybir.dt.float32r

    nc = bacc.Bacc("TRN2", target_bir_lowering=False, debug=False,
                   num_devices=NCORES)

    kv = nc.dram_tensor("kv", [BPC, N, H], f32, kind="ExternalInput")
    q = nc.dram_tensor("q", [BPC, H], f32, kind="ExternalInput")
    ctx_out = nc.dram_tensor("ctx", [BPC, H], f32, kind="ExternalOutput")
    attn_out = nc.dram_tensor("attn", [BPC, N], f32, kind="ExternalOutput")

    with tile.TileContext(nc) as tc:
        with (
            tc.tile_pool(name="kpool", bufs=3) as kpool,
            tc.tile_pool(name="spool", bufs=2) as spool,
            tc.tile_pool(name="s2pool", bufs=2) as s2pool,
            tc.tile_pool(name="qpool", bufs=2) as qpool,
            tc.tile_pool(name="small", bufs=2) as small,
            tc.tile_pool(name="consts", bufs=1) as consts,
            tc.tile_pool(name="pctx", bufs=1, space="PSUM") as pctx,
            tc.tile_pool(name="psmall", bufs=2, space="PSUM") as psmall,
        ):
            ones_col = consts.tile([P, 1], f32)
            nc.vector.memset(ones_col, 1.0)
            ones_row = consts.tile([1, P], f32)
            nc.vector.memset(ones_row, 1.0)

            rep_cm = (tc.For_i(0, repeat, 1) if repeat > 1
                      else contextlib.nullcontext())
            with rep_cm:
                _emit_body(nc, bass, mybir, tc, kpool, spool, s2pool, qpool,
                           small, pctx, psmall, ones_col, ones_row,
                           kv, q, ctx_out, attn_out)

    nc.compile()
    return nc


def _emit_body(nc, bass, mybir, tc, kpool, spool, s2pool, qpool, small, pctx,
               psmall, ones_col, ones_row, kv, q, ctx_out, attn_out):
    f32 = mybir.dt.float32
    f32r = mybir.dt.float32r
    if True:
        if True:
            for b in range(BPC):
                # q[b] broadcast to all 128 partitions: [128, H]
                qb = qpool.tile([P, H], f32)
                q_src = q[b]
                q_bc = bass.AP(tensor=q_src.tensor, offset=q_src.offset,
                               ap=[[0, P]] + list(q_src.ap))
                nc.gpsimd.dma_start(out=qb, in_=q_bc)

                logits = small.tile([P, NT], f32)
                pexp = small.tile([P, NT], f32)
                pexp_r = small.tile([P, NT], f32r)
                ctx_ps = pctx.tile([1, H], mybir.dt.float32)

                for c in range(NCHUNK):
                    # float32r tile (cast during DMA, gpsimd/SWDGE path) so the
                    # full-rate f32r matmul sees a properly "rounded" producer
                    kt = kpool.tile([P, CHUNK_T, H], f32r)
                    src = kv[b, c * CHUNK_T * P:(c + 1) * CHUNK_T * P, :]
                    src = src.rearrange("(t p) h -> p t h", p=P)
                    nc.gpsimd.dma_start(out=kt, in_=src)

                    for t in range(CHUNK_T):
                        i = c * CHUNK_T + t
                        # logits[:, i] = sum_h kt[:, t, h] * qb[:, h] * SCALE
                        # (DVE multiply, then ACT accumulate-reduce)
                        scr = spool.tile([P, H], f32)
                        nc.vector.tensor_mul(scr, kt[:, t, :].bitcast(f32), qb)
                        scr2 = s2pool.tile([P, H], f32)
                        nc.scalar.activation(
                            out=scr2,
                            in_=scr,
                            func=mybir.ActivationFunctionType.Identity,
                            scale=SCALE,
                            accum_out=logits[:, i:i + 1],
                        )

                    i0 = c * CHUNK_T
                    nc.scalar.activation(
                        out=pexp[:, i0:i0 + CHUNK_T],
                        in_=logits[:, i0:i0 + CHUNK_T],
                        func=mybir.ActivationFunctionType.Exp,
                    )
                    nc.scalar.activation(
                        out=pexp_r[:, i0:i0 + CHUNK_T],
                        in_=logits[:, i0:i0 + CHUNK_T],
                        func=mybir.ActivationFunctionType.Exp,
                    )
                    for t in range(CHUNK_T):
                        i = c * CHUNK_T + t
                        # ctx_ps[0, :] += pexp[:, i].T @ kt[:, t, :]
                        for j in range(HT):
                            nc.tensor.matmul(
                                ctx_ps[0:1, j * HB:(j + 1) * HB],
                                pexp_r[:, i:i + 1],
                                kt[:, t, j * HB:(j + 1) * HB],
                                start=(i == 0),
                                stop=(i == NT - 1),
                            )

                # softmax denominator: total = sum over all 4096 exps
                s1 = small.tile([P, 1], f32)
                nc.vector.reduce_sum(out=s1, in_=pexp,
                                     axis=mybir.AxisListType.X)
                tot_ps = psmall.tile([1, 1], mybir.dt.float32)
                nc.tensor.matmul(tot_ps, ones_col, s1, start=True, stop=True)
                tot_sb = small.tile([1, 1], f32)
                nc.scalar.copy(out=tot_sb, in_=tot_ps)
                # broadcast total to 128 partitions via rank-1 matmul
                totbc_ps = psmall.tile([P, 1], mybir.dt.float32)
                nc.tensor.matmul(totbc_ps, ones_row, tot_sb,
                                 start=True, stop=True)
                invbc = small.tile([P, 1], f32)
                nc.vector.reciprocal(invbc, totbc_ps)

                # ctx = ctx_ps * inv
                ctx_sb = small.tile([1, H], f32)
                nc.scalar.mul(out=ctx_sb, in_=ctx_ps, mul=invbc[0:1, :])
                nc.sync.dma_start(out=ctx_out[b:b + 1, :], in_=ctx_sb)

                # attn: transpose pexp [128, 32] -> [32, 128], scale, store
                pT = small.tile([NT, P], f32)
                for k in range(P // NT):
                    nc.vector.transpose(
                        out=pT[:, k * NT:(k + 1) * NT],
                        in_=pexp[k * NT:(k + 1) * NT, :],
                    )
                nc.vector.tensor_scalar_mul(pT, pT, invbc[0:NT, :])
                attn_view = attn_out[b].rearrange("(a m) -> a m", m=P)
                nc.sync.dma_start(out=attn_view, in_=pT)


def _get_nc():
    global _compiled_nc
    if _compiled_nc is None:
        _compiled_nc = _build()
    return _compiled_nc


def kernel(outputs: np.ndarray, last_h: np.ndarray):
    from concourse.bass_utils import run_bass_kernel_spmd

    nc = _get_nc()
    outputs = np.asarray(outputs, dtype=np.float32)
    last_h = np.asarray(last_h, dtype=np.float32)

    in_maps = [
        {"kv": outputs[c * BPC:(c + 1) * BPC],
         "q": last_h[c * BPC:(c + 1) * BPC]}
        for c in range(NCORES)
    ]
    res = run_bass_kernel_spmd(nc, in_maps, list(range(NCORES)))
    ctx = np.concatenate([res.results[c]["ctx"] for c in range(NCORES)], axis=0)
    attn = np.concatenate([res.results[c]["attn"] for c in range(NCORES)],
                          axis=0)
    return ctx, attn


# revision 19
# speedup vs baseline: 42.2554x; 1.2439x over previous
"""Single-query attention (decode step) on 8 Trainium2 NeuronCores.

Problem: outputs [16, 4096, 2048] f32 (K = V), last_h [16, 2048] f32 (q).
Returns (ctx [16, 2048], attn [16, 4096]).

Sharding: batch dim 16 -> 2 batches per core (data parallel, no collectives).
Per core, per batch: stream K tiles [128, 2048] (1 MB DMAs) from HBM ONCE
(the problem is HBM-bandwidth bound: 64 MB/core of f32 K=V data):
  - QK^T: DVE tensor_mul against a partition-broadcast copy of q, then ACT
    Identity-activation with accum_out (free-dim accumulate) -> logits [128,32]
    (the fused DVE tensor_tensor_reduce op crashes this HW's ucode, so the
    multiply and the reduce run on separate engines - both have the slack)
  - softmax without max-subtraction (logits are O(5) for standard-normal
    inputs; exp is safe in fp32); denominator via ones-vector matmul partition
    reduction; normalization applied at the end
  - AV: TensorE matmul, p column [128,1] stationary, K tile moving, dtype
    float32r (TF32-like, full rate; plain fp32 is 4 cycles/row and would make
    PE the bottleneck), accumulated over the 32 n-tiles in PSUM [1, 2048].
    K tiles are cast f32->f32r during the DMA (SWDGE path) because the
    compiler requires f32r matmul inputs to come from an f32r producer.
Engine budget per core: DMA ~190us (bound), DVE ~140us, ACT ~135us, PE ~57us.
"""

import numpy as np

B, N, H = 16, 4096, 2048
NCORES = 8
BPC = B // NCORES        # batches per core
P = 128                  # partitions
NT = N // P              # n-tiles per batch (32)
CHUNK_T = 1              # n-tiles per DMA chunk
NCHUNK = NT // CHUNK_T
HB = 512                 # AV matmul moving free-dim (one PSUM bank of fp32)
HT = H // HB             # h-chunks for AV (4)
SCALE = float(1.0 / np.sqrt(np.float32(H), dtype=np.float32))

_compiled_nc = None


def _build(repeat: int = 1, dma_mode: str = "cast_dma", chunk_t: int = 1,
           kbufs: int = 12, sbufs: int = 2):
    """Build + compile the per-core program. repeat>1 wraps the whole body in
    an on-device For loop (used only for benchmarking; amortizes the host
    dispatch overhead so device time can be measured)."""
    import contextlib
    import concourse.bass as bass
    import concourse.bacc as bacc
    import concourse.tile as tile
    from concourse import mybir

    f32 = mybir.dt.float32
    f32r = mybir.dt.float32r

    nc = bacc.Bacc("TRN2", target_bir_lowering=False, debug=False,
                   num_devices=NCORES)

    kv = nc.dram_tensor("kv", [BPC, N, H], f32, kind="ExternalInput")
    q = nc.dram_tensor("q", [BPC, H], f32, kind="ExternalInput")
    ctx_out = nc.dram_tensor("ctx", [BPC, H], f32, kind="ExternalOutput")
    attn_out = nc.dram_tensor("attn", [BPC, N], f32, kind="ExternalOutput")

    with tile.TileContext(nc) as tc:
        with (
            tc.tile_pool(name="kpool", bufs=kbufs) as kpool,
            tc.tile_pool(name="spool", bufs=sbufs) as spool,
            tc.tile_pool(name="s2pool", bufs=sbufs) as s2pool,
            tc.tile_pool(name="qpool", bufs=2) as qpool,
            tc.tile_pool(name="small", bufs=2) as small,
            tc.tile_pool(name="consts", bufs=1) as consts,
            tc.tile_pool(name="pctx", bufs=1, space="PSUM") as pctx,
            tc.tile_pool(name="psmall", bufs=2, space="PSUM") as psmall,
        ):
            ones_col = consts.tile([P, 1], f32)
            nc.vector.memset(ones_col, 1.0)
            ones_row = consts.tile([1, P], f32)
            nc.vector.memset(ones_row, 1.0)

            rep_cm = (tc.For_i(0, repeat, 1) if repeat > 1
                      else contextlib.nullcontext())
            with rep_cm:
                _emit_body(nc, bass, mybir, tc, kpool, spool, s2pool, qpool,
                           small, pctx, psmall, ones_col, ones_row,
                           kv, q, ctx_out, attn_out, dma_mode, chunk_t)

    nc.compile()
    return nc


def _emit_body(nc, bass, mybir, tc, kpool, spool, s2pool, qpool, small, pctx,
               psmall, ones_col, ones_row, kv, q, ctx_out, attn_out,
               dma_mode="cast_dma", chunk_t=CHUNK_T):
    f32 = mybir.dt.float32
    f32r = mybir.dt.float32r
    if True:
        if True:
            for b in range(BPC):
                # q[b] broadcast to all 128 partitions: [128, H]
                # (DRAM source with 0-step partition dim; row-buffer friendly)
                qb = qpool.tile([P, H], f32)
                q_src = q[b]
                q_bc = bass.AP(tensor=q_src.tensor, offset=q_src.offset,
                               ap=[[0, P]] + list(q_src.ap))
                nc.gpsimd.dma_start(out=qb, in_=q_bc)

                logits = small.tile([P, NT], f32)
                pexp = small.tile([P, NT], f32)
                pexp_r = (small.tile([P, NT], f32r, name="pexp_r")
                          if dma_mode != "f32" else None)
                ctx_ps = pctx.tile([1, H], mybir.dt.float32)

                nchunk = NT // chunk_t
                for c in range(nchunk):
                    src = kv[b, c * chunk_t * P:(c + 1) * chunk_t * P, :]
                    src = src.rearrange("(t p) h -> p t h", p=P)
                    if dma_mode == "dma_only":
                        kt = kpool.tile([P, chunk_t, H], f32r, name="kt",
                                        tag="kt")
                        nc.gpsimd.dma_start(out=kt, in_=src)
                        continue
                    if dma_mode == "dma_only_sync":
                        kt = kpool.tile([P, chunk_t, H], f32, name="kt",
                                        tag="kt")
                        nc.sync.dma_start(out=kt, in_=src)
                        continue
                    if dma_mode == "mixed":
                        # alternate chunks between the SWDGE cast path (f32r)
                        # and the HWDGE sync path (plain f32; those tiles use
                        # the 4-cycle fp32 matmul - PE has the slack)
                        if c % 2 == 0:
                            kt = kpool.tile([P, chunk_t, H], f32r, name="kt",
                                            tag="kt")
                            nc.gpsimd.dma_start(out=kt, in_=src)
                        else:
                            kt = kpool.tile([P, chunk_t, H], f32, name="ktf",
                                            tag="ktf")
                            nc.sync.dma_start(out=kt, in_=src)
                    elif dma_mode == "cast_dma":
                        # float32r tile (cast during DMA, gpsimd/SWDGE path) so
                        # the full-rate f32r matmul sees a "rounded" producer
                        kt = kpool.tile([P, chunk_t, H], f32r, name="kt", tag="kt")
                        nc.gpsimd.dma_start(out=kt, in_=src)
                    elif dma_mode == "sync_copy":
                        kt_f = kpool.tile([P, chunk_t, H], f32, name="kt_f", tag="kt_f")
                        nc.sync.dma_start(out=kt_f, in_=src)
                        kt = kpool.tile([P, chunk_t, H], f32r, name="kt", tag="kt")
                        nc.gpsimd.tensor_copy(out=kt, in_=kt_f)
                    else:  # f32: plain fp32 matmul path
                        kt = kpool.tile([P, chunk_t, H], f32, name="kt", tag="kt")
                        nc.sync.dma_start(out=kt, in_=src)

                    kt_is_f32 = kt.dtype == f32
                    for t in range(chunk_t):
                        i = c * chunk_t + t
                        # logits[:, i] = sum_h kt[:, t, h] * qb[:, h] * SCALE
                        # (DVE multiply, then ACT accumulate-reduce)
                        scr = spool.tile([P, H], f32)
                        kt_f32view = kt[:, t, :] if kt_is_f32 else \
                            kt[:, t, :].bitcast(f32)
                        nc.vector.tensor_mul(scr, kt_f32view, qb)
                        scr2 = s2pool.tile([P, H], f32)
                        nc.scalar.activation(
                            out=scr2,
                            in_=scr,
                            func=mybir.ActivationFunctionType.Identity,
                            scale=SCALE,
                            accum_out=logits[:, i:i + 1],
                        )

                    i0 = c * chunk_t
                    nc.scalar.activation(
                        out=pexp[:, i0:i0 + chunk_t],
                        in_=logits[:, i0:i0 + chunk_t],
                        func=mybir.ActivationFunctionType.Exp,
                    )
                    if dma_mode != "f32":
                        nc.scalar.activation(
                            out=pexp_r[:, i0:i0 + chunk_t],
                            in_=logits[:, i0:i0 + chunk_t],
                            func=mybir.ActivationFunctionType.Exp,
                        )
                    for t in range(chunk_t):
                        i = c * chunk_t + t
                        # ctx_ps[0, :] += pexp[:, i].T @ kt[:, t, :]
                        lhs = (pexp[:, i:i + 1] if kt_is_f32
                               else pexp_r[:, i:i + 1])
                        for j in range(HT):
                            nc.tensor.matmul(
                                ctx_ps[0:1, j * HB:(j + 1) * HB],
                                lhs,
                                kt[:, t, j * HB:(j + 1) * HB],
                                start=(i == 0),
                                stop=(i == NT - 1),
                            )

                if dma_mode.startswith("dma_only"):
                    zt = small.tile([1, H], f32, name="zt")
                    nc.vector.memset(zt, 0.0)
                    nc.sync.dma_start(out=ctx_out[b:b + 1, :], in_=zt)
                    zt2 = small.tile([NT, P], f32, name="zt2")
                    nc.vector.memset(zt2, 0.0)
                    av = attn_out[b].rearrange("(a m) -> a m", m=P)
                    nc.sync.dma_start(out=av, in_=zt2)
                    continue

                # softmax denominator: total = sum over all 4096 exps
                s1 = small.tile([P, 1], f32)
                nc.vector.reduce_sum(out=s1, in_=pexp,
                                     axis=mybir.AxisListType.X)
                tot_ps = psmall.tile([1, 1], mybir.dt.float32)
                nc.tensor.matmul(tot_ps, ones_col, s1, start=True, stop=True)
                tot_sb = small.tile([1, 1], f32)
                nc.scalar.copy(out=tot_sb, in_=tot_ps)
                # broadcast total to 128 partitions via rank-1 matmul
                totbc_ps = psmall.tile([P, 1], mybir.dt.float32)
                nc.tensor.matmul(totbc_ps, ones_row, tot_sb,
                                 start=True, stop=True)
                invbc = small.tile([P, 1], f32)
                nc.vector.reciprocal(invbc, totbc_ps)

                # ctx = ctx_ps * inv
                ctx_sb = small.tile([1, H], f32)
                nc.scalar.mul(out=ctx_sb, in_=ctx_ps, mul=invbc[0:1, :])
                nc.sync.dma_start(out=ctx_out[b:b + 1, :], in_=ctx_sb)

                # attn: transpose pexp [128, 32] -> [32, 128], scale, store
                pT = small.tile([NT, P], f32)
                for k in range(P // NT):
                    nc.vector.transpose(
                        out=pT[:, k * NT:(k + 1) * NT],
                        in_=pexp[k * NT:(k + 1) * NT, :],
                    )
                nc.vector.tensor_scalar_mul(pT, pT, invbc[0:NT, :])
                attn_view = attn_out[b].rearrange("(a m) -> a m", m=P)
                nc.sync.dma_start(out=attn_view, in_=pT)


def _get_nc():
    global _compiled_nc
    if _compiled_nc is None:
        _compiled_nc = _build()
    return _compiled_nc


def kernel(outputs: np.ndarray, last_h: np.ndarray):
    from concourse.bass_utils import run_bass_kernel_spmd

    nc = _get_nc()
    outputs = np.asarray(outputs, dtype=np.float32)
    last_h = np.asarray(last_h, dtype=np.float32)

    in_maps = [
        {"kv": outputs[c * BPC:(c + 1) * BPC],
         "q": last_h[c * BPC:(c + 1) * BPC]}
        for c in range(NCORES)
    ]
    res = run_bass_kernel_spmd(nc, in_maps, list(range(NCORES)))
    ctx = np.concatenate([res.results[c]["ctx"] for c in range(NCORES)], axis=0)
    attn = np.concatenate([res.results[c]["attn"] for c in range(NCORES)],
                          axis=0)
    return ctx, attn
